# revision 16
# baseline (speedup 1.0000x reference)
"""Trainium2 Bass kernel for nn_LocationEncoder (L1-distance attention).

Math (per batch element b):
    key   = ctx_c @ W_ctx + b_ctx                  [C, H]
    query = tgt_c @ W_ctx + b_ctx                  [T, H]
    value = [ctx_c, ctx_y] @ W_in + b_in           [C, H]
    S[t, c]  = sum_h |0.5*(key[c,h] - query[t,h])|  (= -score, always >= 0)
    w = softmax(-S, axis=c);  out = (w @ value) @ W_tgt + b_tgt

Sharding: 8 cores = (4 batches) x (2 halves of T).  Each core handles
T_LOC = 256 targets against the full C = 512 context of its batch.

This execution environment has a large per-instruction dispatch overhead
(~33 us/instruction measured), so the kernel is built from ~37 very large
instructions:
  - key/value via 4 fused matmuls ([key_half | value] share the lhsT),
    nq = -0.5*query via 2 matmuls; nq is bounced through DRAM and
    broadcast-DMA'd to all 128 partitions (q_rep).
  - d[c, t, h] = key_half[c, h] + nq[t, h]: ONE tensor_tensor add per
    128-c chunk with 3D broadcast APs, FD = 256*128 = 32768 elements.
  - S^T[c, t] = sum_h |d|: ONE segmented tensor_reduce (axis=X,
    apply_absolute_value) per chunk.
  - softmax without max-subtraction (scores <= 0 cannot overflow exp):
    one Exp over all chunks, Z row via 4 ones-matmuls, reciprocal,
    broadcast-DMA of 1/Z, one multiply -> normalized weights.
  - rep^T = value.T-style accumulation (4 matmuls), out = rep^T.T @ W_tgt
    (2 matmuls) + b_tgt (one add), one output DMA.
"""

import numpy as np
import ml_dtypes

import concourse.bass as bass
import concourse.mybir as mybir
import concourse.tile as tile
from concourse import bacc
from concourse.bass_utils import run_bass_kernel_spmd

B, C_CTX, T, H = 4, 512, 512, 128
C_SIZE, Y_SIZE = 3, 2
N_CORES = 8
T_LOC = T * B // N_CORES  # 256 targets per core
N_CC = C_CTX // 128  # 4 context chunks
N_TB = T_LOC // 128  # 2 target half-blocks

F32 = mybir.dt.float32
BF16 = mybir.dt.bfloat16
AF = mybir.ActivationFunctionType
ALU = mybir.AluOpType

# blob_f32 column offsets
O_LHS = 0        # [6, 512]  rows: ctx_cT(3), ones(1), ctx_yT(2)
O_TGT = 512      # [4, 256]  rows: tgt_cT(3), ones(1)
O_KV = 768       # [6, 256]  [:, :128]=key rhs (0.5W_ctx,0.5b,0,0) [:,128:]=W_in_ext
O_NQ = 1024      # [4, 128]  -0.5*W_ctx, -0.5*b_ctx
O_BT = 1152      # [128, 128] b_tgt broadcast
F32_COLS = 1280

_CACHE: dict = {}


def _build(reps: int = 1):
    nc = bacc.Bacc("TRN2", target_bir_lowering=False)

    blob_f = nc.dram_tensor("blob_f", [128, F32_COLS], F32,
                            kind="ExternalInput")
    blob_b = nc.dram_tensor("blob_b", [128, H + 1], BF16,
                            kind="ExternalInput")
    nq_dram = nc.dram_tensor("nq_dram", [T_LOC, H], BF16)
    zs_dram = nc.dram_tensor("zs_dram", [1, T_LOC], F32)
    out_d = nc.dram_tensor("out", [H, T_LOC], F32, kind="ExternalOutput")

    with tile.TileContext(nc) as tc:
        with (
            tc.tile_pool(name="consts", bufs=1) as consts,
            tc.tile_pool(name="work", bufs=1) as work,
            tc.tile_pool(name="ps", bufs=1, space="PSUM") as ps,
        ):
            # ---- load constants ----
            sb_f = consts.tile([128, F32_COLS], F32)
            sb_b = consts.tile([128, H + 1], BF16)
            nc.sync.dma_start(out=sb_f[:], in_=blob_f[:])
            nc.sync.dma_start(out=sb_b[:], in_=blob_b[:])
            ones_col = sb_b[:, H:H + 1]

            # ---- projections (kv chunks + nq halves share one cast) ----
            psum_proj = ps.tile([128, N_CC * 256 + N_TB * H], F32, tag="proj")
            for cc in range(N_CC):
                nc.tensor.matmul(
                    psum_proj[:, cc * 256:(cc + 1) * 256],
                    sb_f[0:6, O_LHS + cc * 128:O_LHS + (cc + 1) * 128],
                    sb_f[0:6, O_KV:O_KV + 256],
                    start=True, stop=True)
            for j in range(N_TB):
                nc.tensor.matmul(
                    psum_proj[:, 1024 + j * H:1024 + (j + 1) * H],
                    sb_f[0:4, O_TGT + j * 128:O_TGT + (j + 1) * 128],
                    sb_f[0:4, O_NQ:O_NQ + 128],
                    start=True, stop=True)
            proj_bf = consts.tile([128, N_CC * 256 + N_TB * H], BF16)
            nc.vector.tensor_copy(out=proj_bf[:], in_=psum_proj[:])
            kv_bf = proj_bf[:, 0:1024].rearrange("p (cc n) -> p cc n", n=256)
            nq_bf = proj_bf[:, 1024:1280].rearrange("p (j h) -> p j h", h=H)

            # bounce nq through DRAM, then broadcast to all 128 partitions
            nq_rows = nq_dram[:].rearrange("(j t) h -> t j h", j=N_TB)
            nc.sync.dma_start(out=nq_rows, in_=nq_bf)
            q_rep = consts.tile([128, T_LOC * H], BF16)
            q_flat = nq_dram[:].flatten().partition_broadcast(128)
            nc.sync.dma_start(out=q_rep[:], in_=q_flat)

            for _ in range(reps):
                # ---- scores: d = key_half + nq ; S = sum_h |d| ----
                s_all = work.tile([128, N_CC, T_LOC], F32, tag="s")
                d3 = work.tile([128, T_LOC, H], BF16, tag="d3")
                for cc in range(N_CC):
                    key_b = kv_bf[:, cc, 0:H].unsqueeze(1).broadcast_to(
                        [128, T_LOC, H])
                    nc.vector.tensor_tensor(
                        out=d3[:], in0=key_b,
                        in1=q_rep[:].rearrange("p (t h) -> p t h", h=H),
                        op=ALU.add)
                    nc.vector.tensor_reduce(
                        out=s_all[:, cc, :], in_=d3[:],
                        axis=mybir.AxisListType.X, op=ALU.add,
                        apply_absolute_value=True)

                # ---- softmax (no max-subtraction needed) ----
                eT = work.tile([128, N_CC, T_LOC], BF16, tag="eT")
                nc.scalar.activation(out=eT[:], in_=s_all[:], func=AF.Exp,
                                     scale=-1.0)
                psum_z = ps.tile([1, T_LOC], F32, tag="z")
                z_out = psum_z[:].unsqueeze(1).broadcast_to([1, 2, T_LOC])
                for half in range(2):
                    nc.tensor.matmul(z_out, ones_col,
                                     eT[:, 2 * half:2 * half + 2, :],
                                     start=(half == 0), stop=(half == 1))
                invz = work.tile([1, T_LOC], F32, tag="invz")
                nc.vector.reciprocal(invz[:], psum_z[:])
                invz_rep = work.tile([128, T_LOC], F32, tag="invzr")
                nc.sync.dma_start(out=zs_dram[:], in_=invz[:])
                nc.sync.dma_start(
                    out=invz_rep[:],
                    in_=zs_dram[:].flatten().partition_broadcast(128))
                e_norm = work.tile([128, N_CC, T_LOC], BF16, tag="en")
                nc.vector.tensor_tensor(
                    out=e_norm[:], in0=eT[:],
                    in1=invz_rep[:].unsqueeze(1).broadcast_to(
                        [128, N_CC, T_LOC]),
                    op=ALU.mult)

                # ---- outT[h2, t] = sum_c VW'[c, h2] * w[c, t] ----
                # (VW' = value @ W_tgt + b_tgt folded on host; softmax weights
                #  sum to 1 so the bias passes through exactly)
                psum_o = ps.tile([H, T_LOC], F32, tag="o")
                for cc in range(N_CC):
                    nc.tensor.matmul(psum_o[:],
                                     kv_bf[:, cc, 128:256],
                                     e_norm[:, cc, :],
                                     start=(cc == 0), stop=(cc == N_CC - 1))
                out_sb = work.tile([H, T_LOC], F32, tag="osb")
                nc.vector.tensor_copy(out=out_sb[:], in_=psum_o[:])
                nc.sync.dma_start(out=out_d[:], in_=out_sb[:])

    nc.compile()
    return nc


def _get_nc(reps: int = 1):
    key = f"nc{reps}"
    if key not in _CACHE:
        _CACHE[key] = _build(reps)
    return _CACHE[key]


def _in_maps(context_x, context_y, target_x, W_in, b_in, W_ctx, b_ctx,
             W_tgt, b_tgt):
    f32 = np.float32
    bf16 = ml_dtypes.bfloat16

    blob_b = np.zeros((128, H + 1), bf16)
    blob_b[:, 0:H] = W_tgt.astype(bf16)
    blob_b[:, H] = bf16(1.0)

    w_in_ext = np.zeros((6, H), f32)
    w_in_ext[0:C_SIZE] = W_in[0:C_SIZE]
    w_in_ext[C_SIZE] = b_in
    w_in_ext[C_SIZE + 1:6] = W_in[C_SIZE:]
    vw = w_in_ext @ W_tgt
    vw[C_SIZE] += b_tgt  # rides the ones-row: sum_c w[t,c] = 1
    kv_rhs = np.zeros((6, 256), f32)
    kv_rhs[0:C_SIZE, 0:H] = 0.5 * W_ctx
    kv_rhs[C_SIZE, 0:H] = 0.5 * b_ctx
    kv_rhs[0:6, H:256] = vw

    nq_rhs = np.zeros((4, H), f32)
    nq_rhs[0:C_SIZE] = -0.5 * W_ctx
    nq_rhs[C_SIZE] = -0.5 * b_ctx

    maps = []
    for core in range(N_CORES):
        b = core // 2
        th = core % 2
        blob_f = np.zeros((128, F32_COLS), f32)
        blob_f[0:C_SIZE, O_LHS:O_LHS + C_CTX] = context_x[b, :, :C_SIZE].T
        blob_f[C_SIZE, O_LHS:O_LHS + C_CTX] = 1.0
        blob_f[4:6, O_LHS:O_LHS + C_CTX] = context_y[b].T
        tgt = target_x[b, th * T_LOC:(th + 1) * T_LOC, :C_SIZE]
        blob_f[0:C_SIZE, O_TGT:O_TGT + T_LOC] = tgt.T
        blob_f[C_SIZE, O_TGT:O_TGT + T_LOC] = 1.0
        blob_f[0:6, O_KV:O_KV + 256] = kv_rhs
        blob_f[0:4, O_NQ:O_NQ + H] = nq_rhs
        blob_f[:, O_BT:O_BT + 128] = b_tgt[None, :]
        maps.append({"blob_f": blob_f, "blob_b": blob_b})
    return maps


def kernel(**inputs):
    nc = _get_nc(_CACHE.get("reps", 1))
    maps = _in_maps(**{k: np.asarray(v) for k, v in inputs.items()})
    res = run_bass_kernel_spmd(nc, maps, core_ids=list(range(N_CORES)),
                               **_CACHE.get("run_kwargs", {}))
    _CACHE["last_result"] = res
    out = np.empty((B, T, H), np.float32)
    for core in range(N_CORES):
        b = core // 2
        th = core % 2
        out[b, th * T_LOC:(th + 1) * T_LOC, :] = res.results[core]["out"].T
    return out


# revision 17
# speedup vs baseline: 7.5968x; 7.5968x over previous
"""Trainium2 Bass kernel for nn_LocationEncoder (L1-distance attention).

Math (per batch element b):
    key   = ctx_c @ W_ctx + b_ctx                  [C, H]
    query = tgt_c @ W_ctx + b_ctx                  [T, H]
    value = [ctx_c, ctx_y] @ W_in + b_in           [C, H]
    S[t, c]  = sum_h |0.5*(key[c,h] - query[t,h])|  (= -score, always >= 0)
    w = softmax(-S, axis=c);  out = (w @ value) @ W_tgt + b_tgt

Sharding: 8 cores = (4 batches) x (2 halves of T).  Each core handles
T_LOC = 256 targets against the full C = 512 context of its batch.

This execution environment has a large per-instruction dispatch overhead
(~33 us/instruction measured), so the kernel is built from ~37 very large
instructions:
  - key/value via 4 fused matmuls ([key_half | value] share the lhsT),
    nq = -0.5*query via 2 matmuls; nq is bounced through DRAM and
    broadcast-DMA'd to all 128 partitions (q_rep).
  - d[c, t, h] = key_half[c, h] + nq[t, h]: ONE tensor_tensor add per
    128-c chunk with 3D broadcast APs, FD = 256*128 = 32768 elements.
  - S^T[c, t] = sum_h |d|: ONE segmented tensor_reduce (axis=X,
    apply_absolute_value) per chunk.
  - softmax without max-subtraction (scores <= 0 cannot overflow exp):
    one Exp over all chunks, Z row via 4 ones-matmuls, reciprocal,
    broadcast-DMA of 1/Z, one multiply -> normalized weights.
  - rep^T = value.T-style accumulation (4 matmuls), out = rep^T.T @ W_tgt
    (2 matmuls) + b_tgt (one add), one output DMA.
"""

import numpy as np
import ml_dtypes

import concourse.bass as bass
import concourse.mybir as mybir
import concourse.tile as tile
from concourse import bacc
from concourse.bass_utils import run_bass_kernel_spmd  # noqa: F401

B, C_CTX, T, H = 4, 512, 512, 128
C_SIZE, Y_SIZE = 3, 2
N_CORES = 8
T_LOC = T * B // N_CORES  # 256 targets per core
N_CC = C_CTX // 128  # 4 context chunks
N_TB = T_LOC // 128  # 2 target half-blocks

F32 = mybir.dt.float32
BF16 = mybir.dt.bfloat16
AF = mybir.ActivationFunctionType
ALU = mybir.AluOpType

# blob_f32 column offsets
O_LHS = 0        # [6, 512]  rows: ctx_cT(3), ones(1), ctx_yT(2)
O_TGT = 512      # [4, 256]  rows: tgt_cT(3), ones(1)
O_KV = 768       # [6, 256]  [:, :128]=key rhs (0.5W_ctx,0.5b,0,0) [:,128:]=W_in_ext
O_NQ = 1024      # [4, 128]  -0.5*W_ctx, -0.5*b_ctx
O_BT = 1152      # [128, 128] b_tgt broadcast
F32_COLS = 1280

_CACHE: dict = {}


def _build(reps: int = 1):
    nc = bacc.Bacc("TRN2", target_bir_lowering=False)

    blob_f = nc.dram_tensor("blob_f", [128, F32_COLS], F32,
                            kind="ExternalInput")
    blob_b = nc.dram_tensor("blob_b", [128, H + 1], BF16,
                            kind="ExternalInput")
    nq_dram = nc.dram_tensor("nq_dram", [T_LOC, H], BF16)
    zs_dram = nc.dram_tensor("zs_dram", [1, T_LOC], F32)
    out_d = nc.dram_tensor("out", [H, T_LOC], F32, kind="ExternalOutput")

    with tile.TileContext(nc) as tc:
        with (
            tc.tile_pool(name="consts", bufs=1) as consts,
            tc.tile_pool(name="work", bufs=1) as work,
            tc.tile_pool(name="ps", bufs=1, space="PSUM") as ps,
        ):
            # ---- load constants ----
            sb_f = consts.tile([128, F32_COLS], F32)
            sb_b = consts.tile([128, H + 1], BF16)
            nc.sync.dma_start(out=sb_f[:], in_=blob_f[:])
            nc.sync.dma_start(out=sb_b[:], in_=blob_b[:])
            ones_col = sb_b[:, H:H + 1]

            # ---- projections (kv chunks + nq halves share one cast) ----
            psum_proj = ps.tile([128, N_CC * 256 + N_TB * H], F32, tag="proj")
            for cc in range(N_CC):
                nc.tensor.matmul(
                    psum_proj[:, cc * 256:(cc + 1) * 256],
                    sb_f[0:6, O_LHS + cc * 128:O_LHS + (cc + 1) * 128],
                    sb_f[0:6, O_KV:O_KV + 256],
                    start=True, stop=True)
            for j in range(N_TB):
                nc.tensor.matmul(
                    psum_proj[:, 1024 + j * H:1024 + (j + 1) * H],
                    sb_f[0:4, O_TGT + j * 128:O_TGT + (j + 1) * 128],
                    sb_f[0:4, O_NQ:O_NQ + 128],
                    start=True, stop=True)
            proj_bf = consts.tile([128, N_CC * 256 + N_TB * H], BF16)
            nc.vector.tensor_copy(out=proj_bf[:], in_=psum_proj[:])
            kv_bf = proj_bf[:, 0:1024].rearrange("p (cc n) -> p cc n", n=256)
            nq_bf = proj_bf[:, 1024:1280].rearrange("p (j h) -> p j h", h=H)

            # bounce nq through DRAM, then broadcast to all 128 partitions
            nq_rows = nq_dram[:].rearrange("(j t) h -> t j h", j=N_TB)
            nc.sync.dma_start(out=nq_rows, in_=nq_bf)
            q_rep = consts.tile([128, T_LOC * H], BF16)
            q_flat = nq_dram[:].flatten().partition_broadcast(128)
            nc.sync.dma_start(out=q_rep[:], in_=q_flat)

            for _ in range(reps):
                # ---- scores: d = key_half + nq ; S = sum_h |d| ----
                s_all = work.tile([128, N_CC, T_LOC], F32, tag="s")
                d3 = work.tile([128, T_LOC, H], BF16, tag="d3")
                for cc in range(N_CC):
                    key_b = kv_bf[:, cc, 0:H].unsqueeze(1).broadcast_to(
                        [128, T_LOC, H])
                    nc.vector.tensor_tensor(
                        out=d3[:], in0=key_b,
                        in1=q_rep[:].rearrange("p (t h) -> p t h", h=H),
                        op=ALU.add)
                    nc.vector.tensor_reduce(
                        out=s_all[:, cc, :], in_=d3[:],
                        axis=mybir.AxisListType.X, op=ALU.add,
                        apply_absolute_value=True)

                # ---- softmax (no max-subtraction needed) ----
                eT = work.tile([128, N_CC, T_LOC], BF16, tag="eT")
                nc.scalar.activation(out=eT[:], in_=s_all[:], func=AF.Exp,
                                     scale=-1.0)
                psum_z = ps.tile([1, T_LOC], F32, tag="z")
                z_out = psum_z[:].unsqueeze(1).broadcast_to([1, 2, T_LOC])
                for half in range(2):
                    nc.tensor.matmul(z_out, ones_col,
                                     eT[:, 2 * half:2 * half + 2, :],
                                     start=(half == 0), stop=(half == 1))
                invz = work.tile([1, T_LOC], F32, tag="invz")
                nc.vector.reciprocal(invz[:], psum_z[:])
                invz_rep = work.tile([128, T_LOC], F32, tag="invzr")
                nc.sync.dma_start(out=zs_dram[:], in_=invz[:])
                nc.sync.dma_start(
                    out=invz_rep[:],
                    in_=zs_dram[:].flatten().partition_broadcast(128))
                e_norm = work.tile([128, N_CC, T_LOC], BF16, tag="en")
                nc.vector.tensor_tensor(
                    out=e_norm[:], in0=eT[:],
                    in1=invz_rep[:].unsqueeze(1).broadcast_to(
                        [128, N_CC, T_LOC]),
                    op=ALU.mult)

                # ---- outT[h2, t] = sum_c VW'[c, h2] * w[c, t] ----
                # (VW' = value @ W_tgt + b_tgt folded on host; softmax weights
                #  sum to 1 so the bias passes through exactly)
                psum_o = ps.tile([H, T_LOC], F32, tag="o")
                for cc in range(N_CC):
                    nc.tensor.matmul(psum_o[:],
                                     kv_bf[:, cc, 128:256],
                                     e_norm[:, cc, :],
                                     start=(cc == 0), stop=(cc == N_CC - 1))
                out_sb = work.tile([H, T_LOC], F32, tag="osb")
                nc.vector.tensor_copy(out=out_sb[:], in_=psum_o[:])
                nc.sync.dma_start(out=out_d[:], in_=out_sb[:])

    nc.compile()
    return nc


def _make_runner(nc):
    import jax
    from jax.experimental.shard_map import shard_map
    from jax.sharding import Mesh, PartitionSpec
    from concourse import bass2jax

    bass2jax.install_neuronx_cc_hook()
    partition_name = (nc.partition_id_tensor.name
                      if nc.partition_id_tensor else None)
    in_names, out_names, out_avals, zero_outs = [], [], [], []
    for alloc in nc.m.functions[0].allocations:
        if not isinstance(alloc, mybir.MemoryLocationSet):
            continue
        name = alloc.memorylocations[0].name
        if alloc.kind == "ExternalInput":
            if name != partition_name:
                in_names.append(name)
        elif alloc.kind == "ExternalOutput":
            out_names.append(name)
            shape = tuple(alloc.tensor_shape)
            dtype = mybir.dt.np(alloc.dtype)
            out_avals.append(jax.core.ShapedArray(shape, dtype))
            zero_outs.append(np.zeros(shape, dtype))
    n_params = len(in_names)
    n_outs = len(out_avals)
    all_in = list(in_names) + list(out_names)
    if partition_name is not None:
        all_in.append(partition_name)
    donate = tuple(range(n_params, n_params + n_outs))

    def _body(*args):
        operands = list(args)
        if partition_name is not None:
            operands.append(bass2jax.partition_id_tensor())
        outs = bass2jax._bass_exec_p.bind(
            *operands,
            out_avals=tuple(out_avals),
            in_names=tuple(all_in),
            out_names=tuple(out_names),
            lowering_input_output_aliases=(),
            sim_require_finite=True,
            sim_require_nnan=True,
            nc=nc,
        )
        return tuple(outs)

    devices = jax.devices()[:N_CORES]
    mesh = Mesh(np.asarray(devices), ("core",))
    sharded = jax.jit(
        shard_map(_body, mesh=mesh,
                  in_specs=(PartitionSpec("core"),) * (n_params + n_outs),
                  out_specs=(PartitionSpec("core"),) * n_outs,
                  check_rep=False),
        donate_argnums=donate, keep_unused=True)
    return dict(sharded=sharded, in_names=in_names, out_names=out_names,
                out_avals=out_avals, zero_outs=zero_outs)


def _run_cached(runner, maps):
    concat_in = [np.concatenate([np.asarray(m[name]) for m in maps], axis=0)
                 for name in runner["in_names"]]
    concat_zeros = [np.zeros((N_CORES * z.shape[0], *z.shape[1:]), z.dtype)
                    for z in runner["zero_outs"]]
    out_arrs = runner["sharded"](*concat_in, *concat_zeros)
    out_arrs = [np.asarray(a) for a in out_arrs]
    return [
        {name: out_arrs[i].reshape(N_CORES, *runner["out_avals"][i].shape)[c]
         for i, name in enumerate(runner["out_names"])}
        for c in range(N_CORES)
    ]


def _get_nc(reps: int = 1):
    key = f"nc{reps}"
    if key not in _CACHE:
        _CACHE[key] = _build(reps)
    return _CACHE[key]


def _in_maps(context_x, context_y, target_x, W_in, b_in, W_ctx, b_ctx,
             W_tgt, b_tgt):
    f32 = np.float32
    bf16 = ml_dtypes.bfloat16

    blob_b = np.zeros((128, H + 1), bf16)
    blob_b[:, 0:H] = W_tgt.astype(bf16)
    blob_b[:, H] = bf16(1.0)

    w_in_ext = np.zeros((6, H), f32)
    w_in_ext[0:C_SIZE] = W_in[0:C_SIZE]
    w_in_ext[C_SIZE] = b_in
    w_in_ext[C_SIZE + 1:6] = W_in[C_SIZE:]
    vw = w_in_ext @ W_tgt
    vw[C_SIZE] += b_tgt  # rides the ones-row: sum_c w[t,c] = 1
    kv_rhs = np.zeros((6, 256), f32)
    kv_rhs[0:C_SIZE, 0:H] = 0.5 * W_ctx
    kv_rhs[C_SIZE, 0:H] = 0.5 * b_ctx
    kv_rhs[0:6, H:256] = vw

    nq_rhs = np.zeros((4, H), f32)
    nq_rhs[0:C_SIZE] = -0.5 * W_ctx
    nq_rhs[C_SIZE] = -0.5 * b_ctx

    maps = []
    for core in range(N_CORES):
        b = core // 2
        th = core % 2
        blob_f = np.zeros((128, F32_COLS), f32)
        blob_f[0:C_SIZE, O_LHS:O_LHS + C_CTX] = context_x[b, :, :C_SIZE].T
        blob_f[C_SIZE, O_LHS:O_LHS + C_CTX] = 1.0
        blob_f[4:6, O_LHS:O_LHS + C_CTX] = context_y[b].T
        tgt = target_x[b, th * T_LOC:(th + 1) * T_LOC, :C_SIZE]
        blob_f[0:C_SIZE, O_TGT:O_TGT + T_LOC] = tgt.T
        blob_f[C_SIZE, O_TGT:O_TGT + T_LOC] = 1.0
        blob_f[0:6, O_KV:O_KV + 256] = kv_rhs
        blob_f[0:4, O_NQ:O_NQ + H] = nq_rhs
        blob_f[:, O_BT:O_BT + 128] = b_tgt[None, :]
        maps.append({"blob_f": blob_f, "blob_b": blob_b})
    return maps


def kernel(**inputs):
    reps = _CACHE.get("reps", 1)
    nc = _get_nc(reps)
    rkey = f"runner{reps}"
    if rkey not in _CACHE:
        _CACHE[rkey] = _make_runner(nc)
    maps = _in_maps(**{k: np.asarray(v) for k, v in inputs.items()})
    res = _run_cached(_CACHE[rkey], maps)
    out = np.empty((B, T, H), np.float32)
    for core in range(N_CORES):
        b = core // 2
        th = core % 2
        out[b, th * T_LOC:(th + 1) * T_LOC, :] = res[core]["out"].T
    return out


# revision 20
# speedup vs baseline: 13.7559x; 1.8107x over previous
"""Trainium2 Bass kernel for nn_LocationEncoder (L1-distance attention).

Math (per batch element b):
    key   = ctx_c @ W_ctx + b_ctx                  [C, H]
    query = tgt_c @ W_ctx + b_ctx                  [T, H]
    value = [ctx_c, ctx_y] @ W_in + b_in           [C, H]
    S[t, c]  = sum_h |0.5*(key[c,h] - query[t,h])|  (= -score, always >= 0)
    w = softmax(-S, axis=c);  out = (w @ value) @ W_tgt + b_tgt

Sharding: 8 cores = (4 batches) x (2 halves of T).  Each core handles
T_LOC = 256 targets against the full C = 512 context of its batch.

This execution environment has a large per-instruction dispatch overhead
(~33 us/instruction measured), so the kernel is built from ~37 very large
instructions:
  - key/value via 4 fused matmuls ([key_half | value] share the lhsT),
    nq = -0.5*query via 2 matmuls; nq is bounced through DRAM and
    broadcast-DMA'd to all 128 partitions (q_rep).
  - d[c, t, h] = key_half[c, h] + nq[t, h]: ONE tensor_tensor add per
    128-c chunk with 3D broadcast APs, FD = 256*128 = 32768 elements.
  - S^T[c, t] = sum_h |d|: ONE segmented tensor_reduce (axis=X,
    apply_absolute_value) per chunk.
  - softmax without max-subtraction (scores <= 0 cannot overflow exp):
    one Exp over all chunks, Z row via 4 ones-matmuls, reciprocal,
    broadcast-DMA of 1/Z, one multiply -> normalized weights.
  - rep^T = value.T-style accumulation (4 matmuls), out = rep^T.T @ W_tgt
    (2 matmuls) + b_tgt (one add), one output DMA.
"""

import numpy as np
import ml_dtypes

import concourse.bass as bass
import concourse.mybir as mybir
import concourse.tile as tile
from concourse import bacc
from concourse.bass_utils import run_bass_kernel_spmd  # noqa: F401

B, C_CTX, T, H = 4, 512, 512, 128
C_SIZE, Y_SIZE = 3, 2
N_CORES = 8
T_LOC = T * B // N_CORES  # 256 targets per core
N_CC = C_CTX // 128  # 4 context chunks
N_TB = T_LOC // 128  # 2 target half-blocks
T_DVE = 128  # targets handled by the batched VectorE path (rest: ScalarE+PE)

F32 = mybir.dt.float32
BF16 = mybir.dt.bfloat16
AF = mybir.ActivationFunctionType
ALU = mybir.AluOpType

# blob_f32 column offsets
O_LHS = 0        # [6, 512]  rows: ctx_cT(3), ones(1), ctx_yT(2)
O_TGT = 512      # [4, 256]  rows: tgt_cT(3), ones(1)
O_KV = 768       # [6, 256]  [:, :128]=key rhs (0.5W_ctx,0.5b,0,0) [:,128:]=W_in_ext
O_NQ = 1024      # [4, 128]  -0.5*W_ctx, -0.5*b_ctx
O_BT = 1152      # [128, 128] b_tgt broadcast
F32_COLS = 1280

_CACHE: dict = {}


def _build(reps: int = 1):
    nc = bacc.Bacc("TRN2", target_bir_lowering=False)

    blob_f = nc.dram_tensor("blob_f", [128, F32_COLS], F32,
                            kind="ExternalInput")
    blob_b = nc.dram_tensor("blob_b", [128, H + 1], BF16,
                            kind="ExternalInput")
    nq_dram = nc.dram_tensor("nq_dram", [T_LOC, H], BF16)
    zs_dram = nc.dram_tensor("zs_dram", [1, T_LOC], F32)
    out_d = nc.dram_tensor("out", [H, T_LOC], F32, kind="ExternalOutput")

    with tile.TileContext(nc) as tc:
        with (
            tc.tile_pool(name="consts", bufs=1) as consts,
            tc.tile_pool(name="work", bufs=1) as work,
            tc.tile_pool(name="absd", bufs=6) as absd_pool,
            tc.tile_pool(name="ps", bufs=1, space="PSUM") as ps,
        ):
            # ---- load constants ----
            sb_f = consts.tile([128, F32_COLS], F32)
            sb_b = consts.tile([128, H + 1], BF16)
            nc.sync.dma_start(out=sb_f[:], in_=blob_f[:])
            nc.sync.dma_start(out=sb_b[:], in_=blob_b[:])
            ones_col = sb_b[:, H:H + 1]

            # ---- projections (kv chunks + nq halves + transposed copies
            #      for the per-target ScalarE path, one shared cast) ----
            psum_proj = ps.tile([128, 2048], F32, tag="proj")
            for cc in range(N_CC):
                nc.tensor.matmul(
                    psum_proj[:, cc * 256:(cc + 1) * 256],
                    sb_f[0:6, O_LHS + cc * 128:O_LHS + (cc + 1) * 128],
                    sb_f[0:6, O_KV:O_KV + 256],
                    start=True, stop=True)
            for j in range(N_TB):
                nc.tensor.matmul(
                    psum_proj[:, 1024 + j * H:1024 + (j + 1) * H],
                    sb_f[0:4, O_TGT + j * 128:O_TGT + (j + 1) * 128],
                    sb_f[0:4, O_NQ:O_NQ + 128],
                    start=True, stop=True)
            # keyT_half [h, c] and nqT [h, t] for the ScalarE bias trick
            nc.tensor.matmul(psum_proj[:, 1280:1792],
                             sb_f[0:4, O_KV:O_KV + 128],
                             sb_f[0:4, O_LHS:O_LHS + C_CTX],
                             start=True, stop=True)
            nc.tensor.matmul(psum_proj[:, 1792:2048],
                             sb_f[0:4, O_NQ:O_NQ + 128],
                             sb_f[0:4, O_TGT:O_TGT + T_LOC],
                             start=True, stop=True)
            proj_bf = consts.tile([128, 2048], BF16)
            nc.vector.tensor_copy(out=proj_bf[:], in_=psum_proj[:])
            kv_bf = proj_bf[:, 0:1024].rearrange("p (cc n) -> p cc n", n=256)
            nq_bf = proj_bf[:, 1024:1280].rearrange("p (j h) -> p j h", h=H)
            keyT_bf = proj_bf[:, 1280:1792]
            nqT_f32 = consts.tile([128, T_LOC], F32)
            nc.vector.tensor_copy(out=nqT_f32[:], in_=psum_proj[:, 1792:2048])

            # bounce nq through DRAM, then broadcast to all 128 partitions
            nq_rows = nq_dram[:].rearrange("(j t) h -> t j h", j=N_TB)
            nc.sync.dma_start(out=nq_rows, in_=nq_bf)
            q_rep = consts.tile([128, T_LOC * H], BF16)
            q_flat = nq_dram[:].flatten().partition_broadcast(128)
            nc.sync.dma_start(out=q_rep[:], in_=q_flat)

            for _ in range(reps):
                # ---- scores, split across engines ----
                # t in [0, T_DVE): batched VectorE path
                #   d[c, t, h] = key_half[c, h] + nq[t, h]  (one TT per chunk)
                #   S^T[c, t] = sum_h |d|  (segmented reduce per chunk)
                s_all = work.tile([128, N_CC, T_DVE], F32, tag="s")
                d3 = work.tile([128, T_DVE, H], BF16, tag="d3")
                for cc in range(N_CC):
                    key_b = kv_bf[:, cc, 0:H].unsqueeze(1).broadcast_to(
                        [128, T_DVE, H])
                    nc.vector.tensor_tensor(
                        out=d3[:], in0=key_b,
                        in1=q_rep[:, 0:T_DVE * H].rearrange(
                            "p (t h) -> p t h", h=H),
                        op=ALU.add)
                    nc.vector.tensor_reduce(
                        out=s_all[:, cc, :], in_=d3[:],
                        axis=mybir.AxisListType.X, op=ALU.add,
                        apply_absolute_value=True)

                # t in [T_DVE, T_LOC): per-target ScalarE + TensorE path
                #   absd[h, c] = Abs(keyT_half + (-qT_half[t])), then a
                #   stationary-operand ones-matmul sums over h (partitions)
                psum_sT = ps.tile([128, N_CC, T_LOC - T_DVE], F32, tag="sT")
                for ti, t in enumerate(range(T_DVE, T_LOC)):
                    a = absd_pool.tile([128, C_CTX], BF16, tag="a")
                    nc.scalar.activation(out=a[:], in_=keyT_bf, func=AF.Abs,
                                         bias=nqT_f32[:, t:t + 1], scale=1.0)
                    for cc in range(N_CC):
                        nc.tensor.matmul(psum_sT[:, cc, ti:ti + 1],
                                         a[:, cc * 128:(cc + 1) * 128],
                                         ones_col, start=True, stop=True)

                # ---- softmax (no max-subtraction needed) ----
                eT = work.tile([128, N_CC, T_LOC], BF16, tag="eT")
                nc.scalar.activation(out=eT[:, :, 0:T_DVE], in_=s_all[:],
                                     func=AF.Exp, scale=-1.0)
                nc.scalar.activation(out=eT[:, :, T_DVE:T_LOC],
                                     in_=psum_sT[:], func=AF.Exp, scale=-1.0)
                psum_z = ps.tile([1, T_LOC], F32, tag="z")
                z_out = psum_z[:].unsqueeze(1).broadcast_to([1, 2, T_LOC])
                for half in range(2):
                    nc.tensor.matmul(z_out, ones_col,
                                     eT[:, 2 * half:2 * half + 2, :],
                                     start=(half == 0), stop=(half == 1))
                invz = work.tile([1, T_LOC], F32, tag="invz")
                nc.vector.reciprocal(invz[:], psum_z[:])
                invz_rep = work.tile([128, T_LOC], F32, tag="invzr")
                nc.sync.dma_start(out=zs_dram[:], in_=invz[:])
                nc.sync.dma_start(
                    out=invz_rep[:],
                    in_=zs_dram[:].flatten().partition_broadcast(128))
                e_norm = work.tile([128, N_CC, T_LOC], BF16, tag="en")
                nc.vector.tensor_tensor(
                    out=e_norm[:], in0=eT[:],
                    in1=invz_rep[:].unsqueeze(1).broadcast_to(
                        [128, N_CC, T_LOC]),
                    op=ALU.mult)

                # ---- outT[h2, t] = sum_c VW'[c, h2] * w[c, t] ----
                # (VW' = value @ W_tgt + b_tgt folded on host; softmax weights
                #  sum to 1 so the bias passes through exactly)
                psum_o = ps.tile([H, T_LOC], F32, tag="o")
                for cc in range(N_CC):
                    nc.tensor.matmul(psum_o[:],
                                     kv_bf[:, cc, 128:256],
                                     e_norm[:, cc, :],
                                     start=(cc == 0), stop=(cc == N_CC - 1))
                out_sb = work.tile([H, T_LOC], F32, tag="osb")
                nc.vector.tensor_copy(out=out_sb[:], in_=psum_o[:])
                nc.sync.dma_start(out=out_d[:], in_=out_sb[:])

    nc.compile()
    return nc


def _make_runner(nc):
    import jax
    from jax.experimental.shard_map import shard_map
    from jax.sharding import Mesh, PartitionSpec
    from concourse import bass2jax

    bass2jax.install_neuronx_cc_hook()
    partition_name = (nc.partition_id_tensor.name
                      if nc.partition_id_tensor else None)
    in_names, out_names, out_avals, zero_outs = [], [], [], []
    for alloc in nc.m.functions[0].allocations:
        if not isinstance(alloc, mybir.MemoryLocationSet):
            continue
        name = alloc.memorylocations[0].name
        if alloc.kind == "ExternalInput":
            if name != partition_name:
                in_names.append(name)
        elif alloc.kind == "ExternalOutput":
            out_names.append(name)
            shape = tuple(alloc.tensor_shape)
            dtype = mybir.dt.np(alloc.dtype)
            out_avals.append(jax.core.ShapedArray(shape, dtype))
            zero_outs.append(np.zeros(shape, dtype))
    n_params = len(in_names)
    n_outs = len(out_avals)
    all_in = list(in_names) + list(out_names)
    if partition_name is not None:
        all_in.append(partition_name)
    donate = tuple(range(n_params, n_params + n_outs))

    def _body(*args):
        operands = list(args)
        if partition_name is not None:
            operands.append(bass2jax.partition_id_tensor())
        outs = bass2jax._bass_exec_p.bind(
            *operands,
            out_avals=tuple(out_avals),
            in_names=tuple(all_in),
            out_names=tuple(out_names),
            lowering_input_output_aliases=(),
            sim_require_finite=True,
            sim_require_nnan=True,
            nc=nc,
        )
        return tuple(outs)

    devices = jax.devices()[:N_CORES]
    mesh = Mesh(np.asarray(devices), ("core",))
    sharded = jax.jit(
        shard_map(_body, mesh=mesh,
                  in_specs=(PartitionSpec("core"),) * (n_params + n_outs),
                  out_specs=(PartitionSpec("core"),) * n_outs,
                  check_rep=False),
        donate_argnums=donate, keep_unused=True)
    return dict(sharded=sharded, in_names=in_names, out_names=out_names,
                out_avals=out_avals, zero_outs=zero_outs)


def _run_cached(runner, maps):
    concat_in = [np.concatenate([np.asarray(m[name]) for m in maps], axis=0)
                 for name in runner["in_names"]]
    concat_zeros = [np.zeros((N_CORES * z.shape[0], *z.shape[1:]), z.dtype)
                    for z in runner["zero_outs"]]
    out_arrs = runner["sharded"](*concat_in, *concat_zeros)
    out_arrs = [np.asarray(a) for a in out_arrs]
    return [
        {name: out_arrs[i].reshape(N_CORES, *runner["out_avals"][i].shape)[c]
         for i, name in enumerate(runner["out_names"])}
        for c in range(N_CORES)
    ]


def _get_nc(reps: int = 1):
    key = f"nc{reps}"
    if key not in _CACHE:
        _CACHE[key] = _build(reps)
    return _CACHE[key]


def _in_maps(context_x, context_y, target_x, W_in, b_in, W_ctx, b_ctx,
             W_tgt, b_tgt):
    f32 = np.float32
    bf16 = ml_dtypes.bfloat16

    blob_b = np.zeros((128, H + 1), bf16)
    blob_b[:, 0:H] = W_tgt.astype(bf16)
    blob_b[:, H] = bf16(1.0)

    w_in_ext = np.zeros((6, H), f32)
    w_in_ext[0:C_SIZE] = W_in[0:C_SIZE]
    w_in_ext[C_SIZE] = b_in
    w_in_ext[C_SIZE + 1:6] = W_in[C_SIZE:]
    vw = w_in_ext @ W_tgt
    vw[C_SIZE] += b_tgt  # rides the ones-row: sum_c w[t,c] = 1
    kv_rhs = np.zeros((6, 256), f32)
    kv_rhs[0:C_SIZE, 0:H] = 0.5 * W_ctx
    kv_rhs[C_SIZE, 0:H] = 0.5 * b_ctx
    kv_rhs[0:6, H:256] = vw

    nq_rhs = np.zeros((4, H), f32)
    nq_rhs[0:C_SIZE] = -0.5 * W_ctx
    nq_rhs[C_SIZE] = -0.5 * b_ctx

    maps = []
    for core in range(N_CORES):
        b = core // 2
        th = core % 2
        blob_f = np.zeros((128, F32_COLS), f32)
        blob_f[0:C_SIZE, O_LHS:O_LHS + C_CTX] = context_x[b, :, :C_SIZE].T
        blob_f[C_SIZE, O_LHS:O_LHS + C_CTX] = 1.0
        blob_f[4:6, O_LHS:O_LHS + C_CTX] = context_y[b].T
        tgt = target_x[b, th * T_LOC:(th + 1) * T_LOC, :C_SIZE]
        blob_f[0:C_SIZE, O_TGT:O_TGT + T_LOC] = tgt.T
        blob_f[C_SIZE, O_TGT:O_TGT + T_LOC] = 1.0
        blob_f[0:6, O_KV:O_KV + 256] = kv_rhs
        blob_f[0:4, O_NQ:O_NQ + H] = nq_rhs
        blob_f[:, O_BT:O_BT + 128] = b_tgt[None, :]
        maps.append({"blob_f": blob_f, "blob_b": blob_b})
    return maps


def kernel(**inputs):
    reps = _CACHE.get("reps", 1)
    nc = _get_nc(reps)
    rkey = f"runner{reps}"
    if rkey not in _CACHE:
        _CACHE[rkey] = _make_runner(nc)
    maps = _in_maps(**{k: np.asarray(v) for k, v in inputs.items()})
    res = _run_cached(_CACHE[rkey], maps)
    out = np.empty((B, T, H), np.float32)
    for core in range(N_CORES):
        b = core // 2
        th = core % 2
        out[b, th * T_LOC:(th + 1) * T_LOC, :] = res[core]["out"].T
    return out
